# revision 1
# baseline (speedup 1.0000x reference)
"""AutoregressiveLSTM kernel for 8 Trainium2 NeuronCores.

Strategy (per sharding hint): pure data parallelism — shard the batch dim
(16384) across the 8 cores (2048 each), replicate the tiny LSTM/linear
weights on every core.

Algorithmic reformulation for speed: the reference runs, per outer step t,
one trunk LSTM cell followed by a 4-step autoregressive rollout (5 cells
deep, 96 outer steps = 480 sequential cells). But the rollout chains hang
OFF the trunk: rollout for step t depends only on (h_t, c_t, y_t). So we
run the 96 trunk cells sequentially (phase 1), collecting every (h_t, c_t,
y_t), then run the rollouts for ALL 96 steps together as 4 giant batched
cells over a [96*B_shard, HID] state (phase 2). Identical math, 100 vs 480
sequential steps, and phase 2 runs at full width.
"""

import numpy as np
import jax
import jax.numpy as jnp
from functools import partial

HID = 10
IN = 4
OUT = 4
P = 5
B = 16384
T = 100
TP = T - P + 1  # 96
NDEV = 8
BS = B // NDEV  # 2048 per core


def _cell(xt, h, c, W_ih_T, W_hh_T, b):
    gates = xt @ W_ih_T + h @ W_hh_T + b  # [N, 4H]
    i = jax.nn.sigmoid(gates[:, 0 * HID:1 * HID])
    f = jax.nn.sigmoid(gates[:, 1 * HID:2 * HID])
    g = jnp.tanh(gates[:, 2 * HID:3 * HID])
    o = jax.nn.sigmoid(gates[:, 3 * HID:4 * HID])
    c_new = f * c + i * g
    h_new = o * jnp.tanh(c_new)
    return h_new, c_new


def _shard_fn(x, W_ih_T, W_hh_T, b, W_lin_T, b_lin):
    # x: [BS, T, IN] local shard
    xs = jnp.transpose(x[:, :TP, :], (1, 0, 2))  # [TP, BS, IN]
    h0 = jnp.zeros((BS, HID), x.dtype)
    c0 = jnp.zeros((BS, HID), x.dtype)

    def step(carry, xt):
        h, c = carry
        h, c = _cell(xt, h, c, W_ih_T, W_hh_T, b)
        y = h @ W_lin_T + b_lin
        return (h, c), (h, c, y)

    (hF, cF), (Hs, Cs, Ys) = jax.lax.scan(step, (h0, c0), xs)
    # Hs/Cs: [TP, BS, HID]; Ys: [TP, BS, OUT]

    Hb = Hs.reshape(TP * BS, HID)
    Cb = Cs.reshape(TP * BS, HID)
    Yb = Ys.reshape(TP * BS, OUT)
    outs = [Yb]
    for _ in range(P - 1):
        Hb, Cb = _cell(Yb, Hb, Cb, W_ih_T, W_hh_T, b)
        Yb = Hb @ W_lin_T + b_lin
        outs.append(Yb)
    ys = jnp.stack(outs, axis=1)  # [TP*BS, P, OUT]
    ys = ys.reshape(TP, BS, P, OUT).transpose(1, 0, 2, 3)  # [BS, TP, P, OUT]
    return ys, cF


@partial(jax.pmap, axis_name="d",
         in_axes=(0, None, None, None, None, None),
         out_axes=(0, 0))
def _pmapped(x, W_ih_T, W_hh_T, b, W_lin_T, b_lin):
    return _shard_fn(x, W_ih_T, W_hh_T, b, W_lin_T, b_lin)


def kernel(x, W_ih, W_hh, b_ih, b_hh, W_lin, b_lin):
    x = np.asarray(x, dtype=np.float32)
    W_ih_T = np.ascontiguousarray(np.asarray(W_ih, np.float32).T)   # [IN, 4H]
    W_hh_T = np.ascontiguousarray(np.asarray(W_hh, np.float32).T)   # [HID, 4H]
    b = (np.asarray(b_ih, np.float32) + np.asarray(b_hh, np.float32))
    W_lin_T = np.ascontiguousarray(np.asarray(W_lin, np.float32).T)  # [HID, OUT]
    b_lin = np.asarray(b_lin, np.float32)

    xsh = x.reshape(NDEV, BS, T, IN)
    ys, cF = _pmapped(xsh, W_ih_T, W_hh_T, b, W_lin_T, b_lin)
    ys = np.asarray(ys).reshape(B, TP, P, OUT)
    cF = np.asarray(cF).reshape(B, HID)
    return ys, cF[None]
